# revision 1
# baseline (speedup 1.0000x reference)
"""Multi-head attention (B=2, L=2048, D=1024, H=16) on 8 Trainium2 NeuronCores.

Sharding: tensor-parallel over heads. Core c owns heads 2c, 2c+1, i.e. rows
[128c, 128c+128) of Wq/Wk/Wv and columns [128c, 128c+128) of Wo. Each core
computes Q/K/V projections for its 128 channels over all 4096 tokens,
attention for its 2 heads (both batches), and a partial out-projection
y_c = attnO_c @ Wo[:, sl].T. The host sums the 8 partials and adds bo
(the gather/unshard step).

Device-side layout notes:
- Activations arrive pre-transposed (host): qT/kT/vT are [D, B*L] so the
  contraction dim d lands on SBUF partitions without any on-device transpose.
- Scores are computed transposed (S.T tiles [k,q]) so softmax P.T lands in
  SBUF ready to be the PV matmul's moving operand; softmax-over-partitions is
  avoided by skipping the max-subtraction (scores are ~N(0,1); exp cannot
  overflow fp32) and computing row sums r with ones-matmuls on the PE.
- The key-padding mask folds into the exp: bias is 0 / -30000 per k-token,
  added per-partition by the ACT instruction, so masked keys exp to 0.0.
- PV packs the two heads into one PSUM bank via column tiling; since
  start=True clears has_written for the whole bank, shared banks are
  pre-cleared with a zero dummy matmul and all real matmuls accumulate.
- Normalization 1/r is broadcast across partitions with a small K=33 matmul
  (compute engines cannot move data across partitions).
"""

import os
import sys

for _p in ("/opt/trn_rl_repo", os.path.expanduser("~/.axon_site/_ro/trn_rl_repo")):
    if os.path.isdir(_p) and _p not in sys.path:
        sys.path.insert(0, _p)

import numpy as np

B = 2
L = 2048
D = 1024
T = B * L            # 4096 tokens
E = 128              # channels per core (2 heads x 64)
HD = 64              # head dim
N_CORES = 8
SCALE = 1.0 / 8.0    # 1/sqrt(HD)
MASK_BIAS = -30000.0

N_TT = T // 512      # 8 projection token tiles
N_DC = D // 128      # 8 contraction chunks
N_QT = L // 512      # 4 query tiles per batch
N_KT = L // 128      # 16 key tiles per batch

_cached = {}


def _build_program(has_bq, has_bk, has_bv, reps=1, mm_dt="f32r", in_dt="bf16"):
    import concourse.bacc as bacc
    import concourse.mybir as mybir
    import concourse.tile as tile

    F32 = mybir.dt.float32
    AF = mybir.ActivationFunctionType
    ALU = mybir.AluOpType
    R = mybir.dt.float32r if mm_dt == "f32r" else mybir.dt.float32
    IDT = mybir.dt.bfloat16 if in_dt == "bf16" else F32

    def mm(out, lhsT, rhs, **kw):
        nc.tensor.matmul(out, lhsT, rhs, **kw)

    nc = bacc.Bacc("TRN2", target_bir_lowering=False, debug=False, num_devices=N_CORES)
    RD = mybir.dt.float32r if mm_dt == "f32r" else mybir.dt.float32

    qT = nc.dram_tensor("qT", [D, T], IDT, kind="ExternalInput").ap()
    kT = nc.dram_tensor("kT", [D, T], IDT, kind="ExternalInput").ap()
    vT = nc.dram_tensor("vT", [D, T], IDT, kind="ExternalInput").ap()
    wq = nc.dram_tensor("wq", [D, E], IDT, kind="ExternalInput").ap()
    wk = nc.dram_tensor("wk", [D, E], IDT, kind="ExternalInput").ap()
    wv = nc.dram_tensor("wv", [D, E], IDT, kind="ExternalInput").ap()
    wo = nc.dram_tensor("wo", [E, D], RD, kind="ExternalInput").ap()
    mbd = nc.dram_tensor("mb", [128, B * N_KT], F32, kind="ExternalInput").ap()
    onesd = nc.dram_tensor("ones128", [128, 128], F32, kind="ExternalInput").ap()
    identd = nc.dram_tensor("ident", [128, 128], F32, kind="ExternalInput").ap()
    bias_d = {}
    if has_bq:
        bias_d["q"] = nc.dram_tensor("bq", [128, 1], F32, kind="ExternalInput").ap()
    if has_bk:
        bias_d["k"] = nc.dram_tensor("bk", [128, 1], F32, kind="ExternalInput").ap()
    if has_bv:
        bias_d["v"] = nc.dram_tensor("bv", [64, 2], F32, kind="ExternalInput").ap()
    yd = nc.dram_tensor("y", [T, D], F32, kind="ExternalOutput").ap()

    with tile.TileContext(nc) as tc:
        import contextlib
        with contextlib.ExitStack() as ctx:
            const = ctx.enter_context(tc.tile_pool(name="const", bufs=1))
            big = ctx.enter_context(tc.tile_pool(name="big", bufs=1))
            stg = ctx.enter_context(tc.tile_pool(name="stg", bufs=5))
            work = ctx.enter_context(tc.tile_pool(name="work", bufs=3))
            pt_pool = ctx.enter_context(tc.tile_pool(name="ptp", bufs=3))
            psum = ctx.enter_context(tc.tile_pool(name="psum", bufs=2, space="PSUM"))
            psst = ctx.enter_context(tc.tile_pool(name="psst", bufs=2, space="PSUM"))
            psacc = ctx.enter_context(tc.tile_pool(name="psacc", bufs=2, space="PSUM"))

            # ---- constants / weights ----
            w_sb = {}
            for wi, (nm, src) in enumerate((("q", wq), ("k", wk), ("v", wv))):
                w = const.tile([128, D], IDT, name=f"w{nm}_sb")
                eng = nc.sync if wi % 2 else nc.scalar
                eng.dma_start(w.rearrange("p (c e) -> p c e", c=N_DC),
                              src[:].rearrange("(c p) e -> p c e", p=128))
                w_sb[nm] = w
            wo_sbA = const.tile([64, D], R, name="wo_sbA")
            nc.sync.dma_start(wo_sbA[:], wo[0:64, :])
            wo_sbB = const.tile([64, D], R, name="wo_sbB")
            nc.scalar.dma_start(wo_sbB[:], wo[64:128, :])
            mb_sb = const.tile([128, B * N_KT], F32, name="mb_sb")
            nc.sync.dma_start(mb_sb[:], mbd[:])
            ones_sb = const.tile([128, 128], F32, name="ones_sb")
            nc.sync.dma_start(ones_sb[:], onesd[:])
            ident_sb = const.tile([128, 128], F32, name="ident_sb")
            nc.sync.dma_start(ident_sb[:], identd[:])
            b_sb = {}
            for nm, src in bias_d.items():
                bt = const.tile([128, 2] if nm == "v" else [128, 1], F32,
                                name=f"b{nm}_sb")
                nc.sync.dma_start(bt[:], src[:])
                b_sb[nm] = bt

            # ---- persistent activations (feature-major) ----
            QT = big.tile([128, T], R, name="QT")   # [e, tok]
            KT = big.tile([128, T], R, name="KT")   # [e, tok]
            # V layout per 128-token group g: [V_headA(64) | 1 | V_headB(64) | 1]
            # the ones column rides along in the PV matmul and produces the
            # softmax denominator as psum row 64.
            V = big.tile([128, 32 * 130], R, name="V")
            for g in range(32):
                nc.gpsimd.memset(V.bitcast(F32)[:, g * 130 + 64:g * 130 + 65], 1.0)
                nc.gpsimd.memset(V.bitcast(F32)[:, g * 130 + 129:g * 130 + 130], 1.0)
            OTA = big.tile([64, T], R, name="OTA")  # [e0:64, tok] normalized
            OTB = big.tile([64, T], R, name="OTB")  # [e64:128, tok] normalized
            rr = big.tile([128, 1024], F32, name="rr")  # 1/r at partition 64

            for rep in range(reps):
                srcs = {"q": qT, "k": kT, "v": vT}

                def proj_unit(nm, tt):
                    # one strided DMA lands all 8 contraction chunks
                    # side-by-side: s[p, dc*512 + t] = src[dc*128 + p, tt*512 + t]
                    s = stg.tile([128, 4096], IDT,
                                 name=f"{rep}_stg_{nm}_{tt}", tag="stg")
                    eng = nc.sync if nm != "k" else nc.scalar
                    eng.dma_start(
                        s.rearrange("p (c t) -> p c t", c=N_DC),
                        srcs[nm][:, tt * 512:(tt + 1) * 512]
                        .rearrange("(c p) t -> p c t", p=128))
                    ps = psum.tile([128, 512], F32,
                                   name=f"{rep}_ps_{nm}_{tt}", tag="mm")
                    for dc in range(N_DC):
                        mm(ps[:], w_sb[nm][:, dc * 128:(dc + 1) * 128],
                           s[:, dc * 512:(dc + 1) * 512],
                           start=(dc == 0), stop=(dc == N_DC - 1))
                    if nm in ("q", "k"):
                        dst = (QT if nm == "q" else KT)[:, tt * 512:(tt + 1) * 512]
                        if nm in b_sb:
                            nc.vector.tensor_scalar(dst, ps[:], b_sb[nm][:, 0:1],
                                                    None, ALU.add)
                        else:
                            nc.vector.tensor_copy(dst, ps[:])
                    else:
                        vs = work.tile([128, 512], F32, name=f"{rep}_vs_{tt}",
                                       tag="vs")
                        nc.vector.tensor_copy(vs[:], ps[:])
                        for si in range(4):
                            vtp = psum.tile([128, 128], F32,
                                            name=f"{rep}_vtp_{tt}_{si}", tag="mm")
                            nc.tensor.transpose(vtp[:],
                                                vs[:, si * 128:(si + 1) * 128],
                                                ident_sb[:])
                            g = tt * 4 + si
                            nc.vector.tensor_copy(
                                V[:, g * 130:g * 130 + 64], vtp[:, 0:64])
                            nc.vector.tensor_copy(
                                V[:, g * 130 + 65:g * 130 + 129], vtp[:, 64:128])

                # Software-pipelined attention for one (batch, 512-query tile):
                # scores for k-tile kt+1 are issued on the PE before the PV/r
                # accumulation of k-tile kt, so the PE never stalls on the ACT
                # exp of the tile it just produced.
                def attn_steps(b, qt):
                    q0 = b * L + qt * 512
                    ot = [psacc.tile([65, 512], F32,
                                     name=f"{rep}_ot{h}_{b}_{qt}", tag="acc")
                          for h in range(2)]

                    def scores(kt):
                        k0 = b * L + kt * 128
                        g = b * N_KT + kt
                        s = psst.tile([128, 1024], F32,
                                      name=f"{rep}_st_{b}_{qt}_{kt}", tag="st")
                        for h in range(2):
                            mm(s[:, h * 512:(h + 1) * 512],
                               KT[h * 64:(h + 1) * 64, k0:k0 + 128],
                               QT[h * 64:(h + 1) * 64, q0:q0 + 512],
                               start=True, stop=True)
                        p = pt_pool.tile([128, 1024], R,
                                         name=f"{rep}_pt_{b}_{qt}_{kt}", tag="pt")
                        nc.scalar.activation(p[:], s[:], AF.Exp,
                                             bias=mb_sb[:, g:g + 1],
                                             scale=SCALE)
                        return p

                    def pv(kt, p, last):
                        g = b * N_KT + kt
                        for h in range(2):
                            mm(ot[h][:],
                               V[:, g * 130 + 65 * h: g * 130 + 65 * (h + 1)],
                               p[:, h * 512:(h + 1) * 512],
                               start=(kt == 0), stop=last)

                    pprev = scores(0)
                    for kt in range(1, N_KT):
                        if kt % 4 == 0:
                            yield
                        pcur = scores(kt)
                        pv(kt - 1, pprev, last=False)
                        pprev = pcur
                    pv(N_KT - 1, pprev, last=True)

                    # normalize: OT_h = ot_h[0:64] * broadcast(1 / ot_h[64])
                    with nc.allow_low_precision(reason="feeds f32r matmul"):
                        nc.vector.reciprocal(rr[64:65, 0:512], ot[0][64:65, :])
                        nc.vector.reciprocal(rr[64:65, 512:1024], ot[1][64:65, :])
                    for h, OTh in ((0, OTA), (1, OTB)):
                        bc_ps = psum.tile([64, 512], F32,
                                          name=f"{rep}_bc{h}_{b}_{qt}", tag="mm")
                        mm(bc_ps[:], ones_sb[64:65, 0:64],
                           rr[64:65, h * 512:(h + 1) * 512],
                           start=True, stop=True)
                        bc_sb = work.tile([64, 512], F32,
                                          name=f"{rep}_bcs{h}_{b}_{qt}",
                                          tag=f"bcs{h}")
                        nc.vector.tensor_copy(bc_sb[:], bc_ps[:])
                        dst = OTh[:, q0:q0 + 512]
                        nc.vector.tensor_mul(dst, ot[h][0:64, :], bc_sb[:])
                        if "v" in b_sb:
                            nc.vector.tensor_scalar(dst, dst,
                                                    b_sb["v"][0:64, h:h + 1],
                                                    None, ALU.add)

                def attn(b, qt):
                    for _ in attn_steps(b, qt):
                        pass

                def y_unit(g):
                    yp = psst.tile([128, 1024], F32, name=f"{rep}_yp_{g}", tag="st")
                    for nn in range(2):
                        mm(yp[:, nn * 512:(nn + 1) * 512],
                           OTA[:, g * 128:(g + 1) * 128],
                           wo_sbA[:, nn * 512:(nn + 1) * 512],
                           start=True, stop=False)
                        mm(yp[:, nn * 512:(nn + 1) * 512],
                           OTB[:, g * 128:(g + 1) * 128],
                           wo_sbB[:, nn * 512:(nn + 1) * 512],
                           start=False, stop=True)
                    ys = work.tile([128, 1024], F32, name=f"{rep}_ys_{g}", tag="ys")
                    # during attention ACT is exp-bound: keep copies on DVE;
                    # in the tail both engines are free, so alternate.
                    if g < 22 or g % 2:
                        nc.vector.tensor_copy(ys[:], yp[:])
                    else:
                        nc.scalar.copy(ys[:], yp[:])
                    deng = nc.sync if g % 2 else nc.scalar
                    deng.dma_start(yd[g * 128:(g + 1) * 128, :], ys[:])

                # Interleaved emission: every attention tile is cut into
                # four 4-ktile chunks (generator yields) and exactly one
                # DMA/projection/output unit is emitted per chunk, so the ACT
                # exp stream never starves while the PE does projection work.
                # Emission order is program order: every unit precedes the
                # first chunk that reads its output.
                def units(*specs):
                    for sp in specs:
                        if sp is None:
                            continue
                        kind, a = sp
                        if kind == "p":
                            proj_unit(*a)
                        else:
                            y_unit(a)

                proj_unit("k", 0); proj_unit("q", 0); proj_unit("v", 0)
                P = lambda nm, tt: ("p", (nm, tt))
                Y = lambda g: ("y", g)
                sched = [
                    ((0, 0), [[P("k", 1), P("v", 1)], [P("k", 2), P("v", 2)],
                              [P("k", 3), P("v", 3)], [P("q", 1)]]),
                    ((0, 1), [[P("k", 4)], [P("v", 4)], [P("q", 2)], [P("k", 5)]]),
                    ((0, 2), [[P("v", 5)], [P("q", 3)], [P("k", 6)], [P("v", 6)]]),
                    ((0, 3), [[P("q", 4)], [P("k", 7)], [P("v", 7)], [P("q", 5)]]),
                    ((1, 0), [[P("q", 6), Y(0)], [P("q", 7), Y(1)],
                              [Y(2)], [Y(3)]]),
                    ((1, 1), [[Y(4), Y(5)], [Y(6), Y(7)], [Y(8)], [Y(9)]]),
                    ((1, 2), [[Y(10), Y(11)], [Y(12), Y(13)], [Y(14)], [Y(15)]]),
                    ((1, 3), [[Y(16), Y(17)], [Y(18), Y(19)], [Y(20)], [Y(21)]]),
                ]
                for (b, qt), per in sched:
                    gen = attn_steps(b, qt)
                    for ci in range(4):
                        if ci < 3:
                            next(gen)
                        else:
                            for _ in gen:
                                pass
                        for kind, a in per[ci]:
                            if kind == "p":
                                proj_unit(*a)
                            else:
                                y_unit(a)
                for g in range(22, 32):
                    y_unit(g)

    nc.compile()
    return nc


def _host_prep(q, k, v, mask, Wq, bq, Wk, bk, Wv, bv, Wo, in_dt="bf16"):
    """Build the per-core input maps."""
    import ml_dtypes
    f32 = np.float32
    idt = ml_dtypes.bfloat16 if in_dt == "bf16" else f32
    qT = np.ascontiguousarray(q.reshape(T, D).T.astype(idt))
    kT = np.ascontiguousarray(k.reshape(T, D).T.astype(idt))
    vT = np.ascontiguousarray(v.reshape(T, D).T.astype(idt))
    mb = np.where(mask, f32(MASK_BIAS), f32(0.0)).astype(f32)      # [B, L]
    mb = np.ascontiguousarray(
        np.transpose(mb.reshape(B, N_KT, 128), (2, 0, 1)).reshape(128, B * N_KT))
    ones128 = np.ones((128, 128), f32)
    ident = np.eye(128, dtype=f32)

    in_maps = []
    for c in range(N_CORES):
        sl = slice(c * E, (c + 1) * E)
        m = {
            "qT": qT, "kT": kT, "vT": vT,
            "wq": np.ascontiguousarray(Wq[sl, :].T.astype(idt)),
            "wk": np.ascontiguousarray(Wk[sl, :].T.astype(idt)),
            "wv": np.ascontiguousarray(Wv[sl, :].T.astype(idt)),
            "wo": np.ascontiguousarray(Wo[:, sl].T.astype(f32)),
            "mb": mb, "ones128": ones128, "ident": ident,
        }
        if np.any(bq):
            m["bq"] = np.ascontiguousarray(bq[sl].astype(f32).reshape(128, 1))
        if np.any(bk):
            m["bk"] = np.ascontiguousarray(bk[sl].astype(f32).reshape(128, 1))
        if np.any(bv):
            m["bv"] = np.ascontiguousarray(bv[sl].astype(f32).reshape(2, 64).T)
        in_maps.append(m)
    return in_maps


def _build_floor_program():
    """Near-empty program used to measure the axon dispatch floor."""
    import concourse.bacc as bacc
    import concourse.mybir as mybir
    import concourse.tile as tile
    import contextlib

    F32 = mybir.dt.float32
    nc = bacc.Bacc("TRN2", target_bir_lowering=False, debug=False,
                   num_devices=N_CORES)
    x = nc.dram_tensor("x", [128, 8], F32, kind="ExternalInput").ap()
    y = nc.dram_tensor("yf", [128, 8], F32, kind="ExternalOutput").ap()
    with tile.TileContext(nc) as tc:
        with contextlib.ExitStack() as ctx:
            sb = ctx.enter_context(tc.tile_pool(name="sb", bufs=1))
            t = sb.tile([128, 8], F32, name="t")
            nc.sync.dma_start(t[:], x[:])
            nc.sync.dma_start(y[:], t[:])
    nc.compile()
    return nc


def _make_timed_runner(nc, in_maps):
    """Build a reusable jitted runner for `nc` (no output donation — the
    program writes every output element, so uninit result buffers are fine).
    Returns (run_once() -> per-core outputs as numpy, time_iters(n) -> [sec])."""
    import jax
    import time
    import concourse.mybir as mybir
    from concourse import bass2jax
    from jax.experimental.shard_map import shard_map
    from jax.sharding import Mesh, NamedSharding, PartitionSpec

    bass2jax.install_neuronx_cc_hook()

    partition_name = nc.partition_id_tensor.name if nc.partition_id_tensor else None
    in_names, out_names, out_avals, zero_outs = [], [], [], []
    for alloc in nc.m.functions[0].allocations:
        if not isinstance(alloc, mybir.MemoryLocationSet):
            continue
        name = alloc.memorylocations[0].name
        if alloc.kind == "ExternalInput":
            if name != partition_name:
                in_names.append(name)
        elif alloc.kind == "ExternalOutput":
            shape = tuple(alloc.tensor_shape)
            dtype = mybir.dt.np(alloc.dtype)
            out_names.append(name)
            out_avals.append(jax.core.ShapedArray(shape, dtype))
            zero_outs.append(np.zeros(shape, dtype))
    n_params = len(in_names)
    all_in_names = list(in_names) + list(out_names)
    if partition_name is not None:
        all_in_names.append(partition_name)

    def _body(*args):
        operands = list(args)
        if partition_name is not None:
            operands.append(bass2jax.partition_id_tensor())
        outs = bass2jax._bass_exec_p.bind(
            *operands,
            out_avals=tuple(out_avals),
            in_names=tuple(all_in_names),
            out_names=tuple(out_names),
            lowering_input_output_aliases=(),
            sim_require_finite=True,
            sim_require_nnan=True,
            nc=nc,
        )
        return tuple(outs)

    devices = jax.devices()[:N_CORES]
    mesh = Mesh(np.asarray(devices), ("core",))
    nin = n_params + len(out_names)
    fn = jax.jit(shard_map(_body, mesh=mesh,
                           in_specs=(PartitionSpec("core"),) * nin,
                           out_specs=(PartitionSpec("core"),) * len(out_names),
                           check_rep=False))
    sh = NamedSharding(mesh, PartitionSpec("core"))
    dev_args = [
        jax.device_put(
            np.concatenate([np.asarray(in_maps[c][nm]) for c in range(N_CORES)],
                           axis=0), sh)
        for nm in in_names
    ] + [
        jax.device_put(np.zeros((N_CORES * z.shape[0], *z.shape[1:]), z.dtype), sh)
        for z in zero_outs
    ]

    def run_once():
        outs = fn(*dev_args)
        jax.block_until_ready(outs)
        return [
            {nm: np.asarray(outs[i]).reshape(N_CORES, *out_avals[i].shape)[c]
             for i, nm in enumerate(out_names)}
            for c in range(N_CORES)
        ]

    def time_iters(n):
        ts = []
        for _ in range(n):
            t0 = time.perf_counter()
            jax.block_until_ready(fn(*dev_args))
            ts.append(time.perf_counter() - t0)
        return ts

    _chain_cache = {}

    def _chain_fn(n_chain):
        if n_chain in _chain_cache:
            return _chain_cache[n_chain]

        def _body_chain(*args):
            ins = list(args[:n_params])
            seed = list(args[n_params:])
            for _ in range(n_chain):
                operands = ins + seed
                if partition_name is not None:
                    operands.append(bass2jax.partition_id_tensor())
                seed = list(bass2jax._bass_exec_p.bind(
                    *operands,
                    out_avals=tuple(out_avals),
                    in_names=tuple(all_in_names),
                    out_names=tuple(out_names),
                    lowering_input_output_aliases=(),
                    sim_require_finite=True,
                    sim_require_nnan=True,
                    nc=nc,
                ))
            return tuple(seed)

        f = jax.jit(shard_map(_body_chain, mesh=mesh,
                              in_specs=(PartitionSpec("core"),) * nin,
                              out_specs=(PartitionSpec("core"),) * len(out_names),
                              check_rep=False))
        jax.block_until_ready(f(*dev_args))  # compile + warm
        _chain_cache[n_chain] = f
        return f

    def time_chain(n_chain, reps):
        f = _chain_fn(n_chain)
        ts = []
        for _ in range(reps):
            t0 = time.perf_counter()
            jax.block_until_ready(f(*dev_args))
            ts.append(time.perf_counter() - t0)
        return ts

    return run_once, time_iters, time_chain


def kernel(q, k, v, mask, Wq, bq, Wk, bk, Wv, bv, Wo, bo):
    from concourse.bass_utils import run_bass_kernel_spmd

    q, k, v = (np.asarray(x) for x in (q, k, v))
    mask = np.asarray(mask)
    in_maps = _host_prep(q, k, v, mask, np.asarray(Wq), np.asarray(bq),
                         np.asarray(Wk), np.asarray(bk), np.asarray(Wv),
                         np.asarray(bv), np.asarray(Wo))
    key = (("bq" in in_maps[0]), ("bk" in in_maps[0]), ("bv" in in_maps[0]))
    if key not in _cached:
        _cached[key] = _build_program(*key)
    nc = _cached[key]

    trace = bool(int(os.environ.get("KERNEL_TRACE", "0")))
    res = run_bass_kernel_spmd(nc, in_maps, list(range(N_CORES)), trace=trace)
    kernel.last_results = res

    y = np.zeros((T, D), np.float64)
    for i in range(N_CORES):
        y += res.results[i]["y"].astype(np.float64)
    y = (y + np.asarray(bo).astype(np.float64)).astype(np.float32)
    return y.reshape(B, L, D)



# revision 32
# speedup vs baseline: 1.2859x; 1.2859x over previous
"""Multi-head attention (B=2, L=2048, D=1024, H=16) on 8 Trainium2 NeuronCores.

Sharding: tensor-parallel over heads. Core c owns heads 2c, 2c+1, i.e. rows
[128c, 128c+128) of Wq/Wk/Wv and columns [128c, 128c+128) of Wo. Each core
computes Q projections for its 128 channels over all 4096 tokens, K/V
projections over the COMPACTED key set (see below), attention for its 2 heads
(both batches), and a partial out-projection y_c = attnO_c @ Wo[:, sl].T.
The host sums the 8 partials and adds bo (the gather/unshard step).

Key-padding-mask compaction: the mask is known on the host, and masked keys
contribute exactly 0 to softmax numerator and denominator (exp(-inf) == 0).
The host gathers only the unmasked key/value tokens per batch (~L/2 of them),
padding each batch to nk*128 tokens with zero columns that carry a -30000
exp-bias so they also contribute 0. This halves the dominant device work:
score matmuls, PV matmuls, and the softmax exp stream.

Device-side layout notes:
- The host pre-packs activations into the exact SBUF staging layout
  (128 partitions x [tile, chunk, token]) and weights into [128, c*128+e],
  so every load DMA is fully contiguous (full HBM bandwidth, one descriptor
  per partition) — no on-device transposes of inputs.
- Scores are computed transposed (S.T tiles [k,q]) so softmax P.T lands in
  SBUF ready to be the PV matmul's moving operand; softmax-over-partitions is
  avoided by skipping the max-subtraction (scores are ~N(0,1); exp cannot
  overflow fp32) and computing row sums r with a ones-column that rides along
  in the PV matmul (psum row 64 of each head's accumulator).
- The pad-token bias is 0 / -30000 per k-token, added per-partition by the
  ACT instruction, so pad keys exp to 0.0.
- One global software pipeline runs scores two k-tiles ahead of PV across
  attention-tile boundaries, so the exp stream never drains; projection and
  out-projection units are interleaved at fixed slots to fill the PE.
- The ACT (scalar) queue carries ONLY the exp stream; all DMAs are issued
  from the sync queue so a descriptor-generation stall never delays exp.
- Normalization 1/r is broadcast across partitions with a small matmul
  (compute engines cannot move data across partitions); the accumulators are
  copied out of PSUM immediately so the next tile's PV can reuse the banks.
- The two heads' normalized outputs are stacked into one [128, T] tile OT
  (head B's half moved with a SBUF->SBUF DMA, the only engine that can shift
  partitions), so the out-projection contracts all 128 channels in a single
  matmul per 512 output columns — half the PE rows of a split contraction.
- Output partials are written in bf16 (host sums in float64), halving the
  output DMA traffic.
"""

import os
import sys

for _p in ("/opt/trn_rl_repo", os.path.expanduser("~/.axon_site/_ro/trn_rl_repo")):
    if os.path.isdir(_p) and _p not in sys.path:
        sys.path.insert(0, _p)

import numpy as np

B = 2
L = 2048
D = 1024
T = B * L            # 4096 tokens
E = 128              # channels per core (2 heads x 64)
HD = 64              # head dim
N_CORES = 8
SCALE = 1.0 / 8.0    # 1/sqrt(HD)
MASK_BIAS = -30000.0

N_DC = D // 128      # 8 contraction chunks
N_QT = L // 512      # 4 query tiles per batch

_cached = {}
_last_key = None


def _kv_tiles(TKV):
    return [(o, min(512, TKV - o)) for o in range(0, TKV, 512)]


def _build_program(nk, has_bq, has_bk, has_bv, reps=1, mm_dt="f32r",
                   in_dt="bf16", out_dt="bf16"):
    import concourse.bacc as bacc
    import concourse.mybir as mybir
    import concourse.tile as tile

    KB = nk * 128            # padded kv tokens per batch
    TKV = B * KB             # total kv tokens
    G = B * nk               # 128-token kv groups
    kv_tiles = _kv_tiles(TKV)
    # interleave-slot boundaries: after scores(kt) for kt < bounds[ci]
    chunk_bounds = [c for c in ([3] + list(range(5, nk, 2))) if c < nk] + [nk]

    F32 = mybir.dt.float32
    AF = mybir.ActivationFunctionType
    ALU = mybir.AluOpType
    R = mybir.dt.float32r if mm_dt == "f32r" else mybir.dt.float32
    IDT = mybir.dt.bfloat16 if in_dt == "bf16" else F32
    ODT = mybir.dt.bfloat16 if out_dt == "bf16" else F32

    nc = bacc.Bacc("TRN2", target_bir_lowering=False, debug=False,
                   num_devices=N_CORES)

    def mm(out, lhsT, rhs, **kw):
        nc.tensor.matmul(out, lhsT, rhs, **kw)

    # activations pre-packed by the host into the staging layout
    qS = nc.dram_tensor("qS", [128, N_DC * T], IDT, kind="ExternalInput").ap()
    kS = nc.dram_tensor("kS", [128, N_DC * TKV], IDT, kind="ExternalInput").ap()
    vS = nc.dram_tensor("vS", [128, N_DC * TKV], IDT, kind="ExternalInput").ap()
    # weights pre-packed as w[p, c*128+e] = W.T[c*128+p, e]
    wq = nc.dram_tensor("wq", [128, D], IDT, kind="ExternalInput").ap()
    wk = nc.dram_tensor("wk", [128, D], IDT, kind="ExternalInput").ap()
    wv = nc.dram_tensor("wv", [128, D], IDT, kind="ExternalInput").ap()
    wo = nc.dram_tensor("wo", [E, D], R, kind="ExternalInput").ap()
    mbd = nc.dram_tensor("mb", [128, G], F32, kind="ExternalInput").ap()
    onesd = nc.dram_tensor("ones128", [128, 128], F32, kind="ExternalInput").ap()
    identd = nc.dram_tensor("ident", [128, 128], F32, kind="ExternalInput").ap()
    bias_d = {}
    if has_bq:
        bias_d["q"] = nc.dram_tensor("bq", [128, 1], F32, kind="ExternalInput").ap()
    if has_bk:
        bias_d["k"] = nc.dram_tensor("bk", [128, 1], F32, kind="ExternalInput").ap()
    if has_bv:
        bias_d["v"] = nc.dram_tensor("bv", [64, 2], F32, kind="ExternalInput").ap()
    yd = nc.dram_tensor("y", [T, D], ODT, kind="ExternalOutput").ap()

    with tile.TileContext(nc) as tc:
        import contextlib
        with contextlib.ExitStack() as ctx:
            const = ctx.enter_context(tc.tile_pool(name="const", bufs=1))
            big = ctx.enter_context(tc.tile_pool(name="big", bufs=1))
            stg = ctx.enter_context(tc.tile_pool(name="stg", bufs=6))
            work = ctx.enter_context(tc.tile_pool(name="work", bufs=3))
            pt_pool = ctx.enter_context(tc.tile_pool(name="ptp", bufs=3))
            # PSUM budget (8 banks of 2KB/partition):
            #   psst  (scores [128,1024])           2 bufs -> 4 banks
            #   psps  (proj ps/vtp/bc/y [128,512])  2 bufs -> 2 banks
            #   psacc (ot accumulators [65,512])    2 bufs -> 2 banks
            psst = ctx.enter_context(tc.tile_pool(name="psst", bufs=2, space="PSUM"))
            psps = ctx.enter_context(tc.tile_pool(name="psps", bufs=2, space="PSUM"))
            psacc = ctx.enter_context(tc.tile_pool(name="psacc", bufs=2, space="PSUM"))

            # ---- constants / weights (late-needed consts are loaded in the
            # rep-0 prologue so they stay off the startup critical path) ----
            w_sb = {}
            for nm, src in (("k", wk), ("q", wq), ("v", wv)):
                w_sb[nm] = const.tile([128, D], IDT, name=f"w{nm}_sb")
            nc.sync.dma_start(w_sb["k"][:], wk[:])
            nc.sync.dma_start(w_sb["q"][:], wq[:])
            mb_sb = const.tile([128, G], F32, name="mb_sb")
            b_sb = {}
            for nm in bias_d:
                b_sb[nm] = const.tile([128, 2] if nm == "v" else [128, 1], F32,
                                      name=f"b{nm}_sb")
            wo_sb = const.tile([128, D], R, name="wo_sb")
            ones_sb = const.tile([128, 128], F32, name="ones_sb")
            ident_sb = const.tile([128, 128], F32, name="ident_sb")

            # ---- persistent activations (feature-major) ----
            QT = big.tile([128, T], R, name="QT")     # [e, tok]
            KT = big.tile([128, TKV], R, name="KT")   # [e, tok]
            # V layout per 128-token group g: [V_headA(64) | 1 | V_headB(64) | 1]
            V = big.tile([128, G * 130], R, name="V")
            OT = big.tile([128, T], R, name="OT")     # [e, tok] normalized
            rr = big.tile([128, 2048], F32, name="rr")  # 1/r at partition 64

            for rep in range(reps):
                srcs = {"q": qS, "k": kS, "v": vS}
                staged = {}

                def tile_geom(kind, i):
                    if kind == "q":
                        return i * 512, 512
                    return kv_tiles[i]

                def stage(kind, i, eng=None):
                    if (kind, i) in staged:
                        return
                    off, tw = tile_geom(kind, i)
                    s = stg.tile([128, N_DC * tw], IDT,
                                 name=f"{rep}_stg_{kind}_{i}", tag="stg",
                                 padded_shape=[128, N_DC * 512])
                    (eng or nc.sync).dma_start(
                        s[:], srcs[kind][:, N_DC * off:N_DC * (off + tw)])
                    staged[(kind, i)] = s

                def proj_unit(kind, i):
                    stage(kind, i)
                    s = staged.pop((kind, i))
                    off, tw = tile_geom(kind, i)
                    ps = psps.tile([128, tw], F32,
                                   name=f"{rep}_ps_{kind}_{i}", tag="ps",
                                   padded_shape=[128, 512])
                    for dc in range(N_DC):
                        mm(ps[:], w_sb[kind][:, dc * 128:(dc + 1) * 128],
                           s[:, dc * tw:(dc + 1) * tw],
                           start=(dc == 0), stop=(dc == N_DC - 1))
                    if kind in ("q", "k"):
                        dst = (QT if kind == "q" else KT)[:, off:off + tw]
                        if kind in b_sb:
                            nc.vector.tensor_scalar(dst, ps[:], b_sb[kind][:, 0:1],
                                                    None, ALU.add)
                        else:
                            nc.vector.tensor_copy(dst, ps[:])
                    else:
                        vs = work.tile([128, tw], F32, name=f"{rep}_vs_{i}",
                                       tag="vs", padded_shape=[128, 512])
                        nc.vector.tensor_copy(vs[:], ps[:])
                        for si in range(tw // 128):
                            vtp = psps.tile([128, 128], F32,
                                            name=f"{rep}_vtp_{i}_{si}", tag="ps",
                                            padded_shape=[128, 512])
                            nc.tensor.transpose(vtp[:],
                                                vs[:, si * 128:(si + 1) * 128],
                                                ident_sb[:])
                            g = off // 128 + si
                            nc.vector.tensor_copy(
                                V[:, g * 130:g * 130 + 64], vtp[:, 0:64])
                            nc.vector.tensor_copy(
                                V[:, g * 130 + 65:g * 130 + 129], vtp[:, 64:128])

                def y_unit(g, tail=False):
                    ys = work.tile([128, 1024], ODT, name=f"{rep}_ys_{g}", tag="ys")
                    for nn in range(2):
                        yp = psps.tile([128, 512], F32,
                                       name=f"{rep}_yp_{g}_{nn}", tag="ps")
                        mm(yp[:],
                           OT[:, g * 128:(g + 1) * 128],
                           wo_sb[:, nn * 512:(nn + 1) * 512],
                           start=True, stop=True)
                        dst = ys[:, nn * 512:(nn + 1) * 512]
                        # PSUM can only be drained by DVE/ACT; keep ACT clear
                        # for exp except in the tail where exp is done
                        if tail and nn:
                            nc.scalar.copy(dst, yp[:])
                        else:
                            nc.vector.tensor_copy(dst, yp[:])
                    nc.sync.dma_start(yd[g * 128:(g + 1) * 128, :], ys[:])

                # ---- global software-pipelined attention ----
                attn_order = [(b, qt) for b in range(B) for qt in range(N_QT)]
                ot_tiles = {}

                def scores(ai, b, qt, kt):
                    q0 = b * L + qt * 512
                    k0 = b * KB + kt * 128
                    g = b * nk + kt
                    s = psst.tile([128, 1024], F32,
                                  name=f"{rep}_st_{ai}_{kt}", tag="st")
                    for h in range(2):
                        mm(s[:, h * 512:(h + 1) * 512],
                           KT[h * 64:(h + 1) * 64, k0:k0 + 128],
                           QT[h * 64:(h + 1) * 64, q0:q0 + 512],
                           start=True, stop=True)
                    p = pt_pool.tile([128, 1024], R,
                                     name=f"{rep}_pt_{ai}_{kt}", tag="pt")
                    nc.scalar.activation(p[:], s[:], AF.Exp,
                                         bias=mb_sb[:, g:g + 1],
                                         scale=SCALE)
                    return p

                def pv(ai, b, qt, kt, p):
                    if kt == 0:
                        ot_tiles[ai] = [
                            psacc.tile([65, 512], F32,
                                       name=f"{rep}_ot{h}_{ai}", tag="acc")
                            for h in range(2)]
                    ot = ot_tiles[ai]
                    g = b * nk + kt
                    for h in range(2):
                        mm(ot[h][:],
                           V[:, g * 130 + 65 * h: g * 130 + 65 * (h + 1)],
                           p[:, h * 512:(h + 1) * 512],
                           start=(kt == 0), stop=(kt == nk - 1))
                    if kt == nk - 1:
                        norm_begin(ai, b, qt)

                norm_pend = []

                def norm_begin(ai, b, qt):
                    # free the PSUM accumulators ASAP: pull |r| and the head
                    # outputs into SBUF right after the last PV
                    ot = ot_tiles.pop(ai)
                    rb = (ai % 2) * 1024
                    otc = work.tile([64, 1024], F32,
                                    name=f"{rep}_otc_{ai}", tag="otc")
                    with nc.allow_low_precision(reason="feeds f32r matmul"):
                        for h in range(2):
                            nc.vector.reciprocal(
                                rr[64:65, rb + h * 512:rb + (h + 1) * 512],
                                ot[h][64:65, :])
                            nc.vector.tensor_copy(
                                otc[:, h * 512:(h + 1) * 512], ot[h][0:64, :])
                    norm_pend.append((ai, b, qt, otc))

                def norm_finish():
                    ai, b, qt, otc = norm_pend.pop(0)
                    q0 = b * L + qt * 512
                    rb = (ai % 2) * 1024
                    bcs = work.tile([64, 1024], F32,
                                    name=f"{rep}_bcs_{ai}", tag="bcs")
                    for h in range(2):
                        bc_ps = psps.tile([64, 512], F32,
                                          name=f"{rep}_bc{h}_{ai}", tag="ps",
                                          padded_shape=[128, 512])
                        mm(bc_ps[:], ones_sb[64:65, 0:64],
                           rr[64:65, rb + h * 512:rb + (h + 1) * 512],
                           start=True, stop=True)
                        nc.vector.tensor_copy(bcs[:, h * 512:(h + 1) * 512],
                                              bc_ps[:])
                    # head A lands directly on partitions 0:64
                    dstA = OT[0:64, q0:q0 + 512]
                    nc.vector.tensor_mul(dstA, otc[:, 0:512], bcs[:, 0:512])
                    if "v" in b_sb:
                        nc.vector.tensor_scalar(dstA, dstA, b_sb["v"][0:64, 0:1],
                                                None, ALU.add)
                    # head B needs a partition shift 0:64 -> 64:128 (DMA only)
                    otn = work.tile([64, 512], R,
                                    name=f"{rep}_otn_{ai}", tag="otn")
                    nc.vector.tensor_mul(otn[:], otc[:, 512:1024],
                                         bcs[:, 512:1024])
                    if "v" in b_sb:
                        nc.vector.tensor_scalar(otn[:], otn[:],
                                                b_sb["v"][0:64, 1:2],
                                                None, ALU.add)
                    nc.sync.dma_start(OT[64:128, q0:q0 + 512], otn[:])
                    completed_attn[0] = ai + 1

                # ---- dependency-aware interleaved emission ----
                emitted = set()

                def emit(u):
                    if u in emitted:
                        return
                    emitted.add(u)
                    if u[0] == "y":
                        y_unit(u[1])
                    else:
                        proj_unit(u[0], u[1])

                def ensure_kv(colbound):
                    for i, (off, tw) in enumerate(kv_tiles):
                        if off < colbound:
                            emit(("k", i))
                            emit(("v", i))

                prio = []
                nkv = len(kv_tiles)
                qv = [("q", i) for i in range(1, 8)]
                kvv = [x for i in range(1, nkv) for x in (("k", i), ("v", i))]
                while qv or kvv:
                    if kvv:
                        prio.append(kvv.pop(0))
                        prio.append(kvv.pop(0))
                    if qv:
                        prio.append(qv.pop(0))
                for g in range(32):
                    prio.append(("y", g))

                completed_attn = [0]

                def pump(proj_cap=1, y_cap=2):
                    np_, ny = 0, 0
                    picks = []
                    for u in prio:
                        if u in emitted:
                            continue
                        if u[0] == "y":
                            # one-tile delay so the y matmul's OT stationary
                            # never waits on the just-emitted normalize chain
                            if u[1] // 4 >= completed_attn[0] - 1:
                                break
                            if ny < y_cap and np_ + ny < 2:
                                picks.append(u)
                                ny += 1
                        elif np_ < proj_cap:
                            picks.append(u)
                            np_ += 1
                        if np_ + ny >= 2:
                            break
                    for u in picks:
                        prio.remove(u)
                        emit(u)
                    # prefetch staging for upcoming projection units
                    ahead = 0
                    for u in prio:
                        if u[0] in ("k", "v", "q") and u not in emitted:
                            stage(u[0], u[1])
                            ahead += 1
                            if ahead >= 2:
                                break

                def slot():
                    while norm_pend:
                        norm_finish()
                    pump()

                # startup: k0/q0/v0 issue in parallel on three queues; the
                # late-needed consts follow on the scalar queue (idle before
                # the first exp)
                stage("k", 0)
                stage("q", 0, eng=nc.scalar)
                stage("v", 0, eng=nc.gpsimd)
                if rep == 0:
                    nc.scalar.dma_start(w_sb["v"][:], wv[:])
                    nc.scalar.dma_start(mb_sb[:], mbd[:])
                    for nm, src in bias_d.items():
                        nc.scalar.dma_start(b_sb[nm][:], src[:])
                    nc.scalar.dma_start(ident_sb[:], identd[:])
                    nc.scalar.dma_start(ones_sb[:], onesd[:])
                    nc.scalar.dma_start(wo_sb[:], wo[:])
                    for g in range(G):
                        nc.gpsimd.memset(
                            V.bitcast(F32)[:, g * 130 + 64:g * 130 + 65], 1.0)
                        nc.gpsimd.memset(
                            V.bitcast(F32)[:, g * 130 + 129:g * 130 + 130], 1.0)

                DEPTH = 2
                pend = []

                def flush_pv():
                    a = pend.pop(0)
                    pv(*a)

                for ai, (b, qt) in enumerate(attn_order):
                    emit(("q", b * N_QT + qt))
                    lo = 0
                    for bound in chunk_bounds:
                        ensure_kv(b * KB + bound * 128)
                        for kt in range(lo, bound):
                            pend.append((ai, b, qt, kt, scores(ai, b, qt, kt)))
                            if len(pend) > DEPTH:
                                flush_pv()
                        lo = bound
                        slot()
                while pend:
                    flush_pv()
                slot()
                # drain the remaining y units in the tail
                for u in prio:
                    if u not in emitted:
                        if u[0] == "y":
                            emitted.add(u)
                            y_unit(u[1], tail=True)
                        else:
                            emit(u)

    nc.compile()
    return nc


def _host_prep(q, k, v, mask, Wq, bq, Wk, bk, Wv, bv, Wo, in_dt="bf16"):
    """Build the per-core input maps. Returns (in_maps, nk)."""
    import ml_dtypes
    f32 = np.float32
    idt = ml_dtypes.bfloat16 if in_dt == "bf16" else f32

    unm = ~np.asarray(mask)
    U = unm.sum(axis=1).astype(np.int64)          # unmasked keys per batch
    nk = int(max(1, -(-int(U.max()) // 128)))
    KB = nk * 128
    TKV = B * KB
    G = B * nk

    def pack_act(x_tok):
        """[ntok, D] -> [128, N_DC*ntok] staging layout: tiles of <=512
        tokens, inner [chunk, token] per partition."""
        ntok = x_tok.shape[0]
        blocks = []
        for off in range(0, ntok, 512):
            tw = min(512, ntok - off)
            blk = x_tok[off:off + tw].reshape(tw, N_DC, 128)
            blocks.append(np.transpose(blk, (2, 1, 0)).reshape(128, N_DC * tw))
        return np.ascontiguousarray(np.concatenate(blocks, axis=1).astype(idt))

    qS = pack_act(np.asarray(q).reshape(T, D))
    kc = np.zeros((TKV, D), f32)
    vc = np.zeros((TKV, D), f32)
    for b in range(B):
        idx = np.nonzero(unm[b])[0]
        kc[b * KB:b * KB + len(idx)] = k[b, idx]
        vc[b * KB:b * KB + len(idx)] = v[b, idx]
    kS = pack_act(kc)
    vS = pack_act(vc)

    mb = np.full((128, G), f32(MASK_BIAS), f32)
    for b in range(B):
        for t in range(nk):
            n = min(128, max(0, int(U[b]) - t * 128))
            mb[:n, b * nk + t] = 0.0
    ones128 = np.ones((128, 128), f32)
    ident = np.eye(128, dtype=f32)

    def pack_w(W_slice):
        """[E, D] torch-layout slice -> [128, c*128+e] = W.T[c*128+p, e]."""
        wT = W_slice.T.reshape(N_DC, 128, E)          # [c, p, e]
        return np.ascontiguousarray(
            np.transpose(wT, (1, 0, 2)).reshape(128, N_DC * E).astype(idt))

    in_maps = []
    for c in range(N_CORES):
        sl = slice(c * E, (c + 1) * E)
        m = {
            "qS": qS, "kS": kS, "vS": vS,
            "wq": pack_w(Wq[sl, :]),
            "wk": pack_w(Wk[sl, :]),
            "wv": pack_w(Wv[sl, :]),
            "wo": np.ascontiguousarray(Wo[:, sl].T.astype(f32)),
            "mb": mb, "ones128": ones128, "ident": ident,
        }
        if np.any(bq):
            m["bq"] = np.ascontiguousarray(bq[sl].astype(f32).reshape(128, 1))
        if np.any(bk):
            m["bk"] = np.ascontiguousarray(bk[sl].astype(f32).reshape(128, 1))
        if np.any(bv):
            m["bv"] = np.ascontiguousarray(bv[sl].astype(f32).reshape(2, 64).T)
        in_maps.append(m)
    return in_maps, nk


def _make_timed_runner(nc, in_maps):
    """Build a reusable jitted runner for `nc` (no output donation — the
    program writes every output element, so uninit result buffers are fine).
    Returns (run_once() -> per-core outputs as numpy, time_iters(n) -> [sec])."""
    import jax
    import time
    import concourse.mybir as mybir
    from concourse import bass2jax
    from jax.experimental.shard_map import shard_map
    from jax.sharding import Mesh, NamedSharding, PartitionSpec

    bass2jax.install_neuronx_cc_hook()

    partition_name = nc.partition_id_tensor.name if nc.partition_id_tensor else None
    in_names, out_names, out_avals, zero_outs = [], [], [], []
    for alloc in nc.m.functions[0].allocations:
        if not isinstance(alloc, mybir.MemoryLocationSet):
            continue
        name = alloc.memorylocations[0].name
        if alloc.kind == "ExternalInput":
            if name != partition_name:
                in_names.append(name)
        elif alloc.kind == "ExternalOutput":
            shape = tuple(alloc.tensor_shape)
            dtype = mybir.dt.np(alloc.dtype)
            out_names.append(name)
            out_avals.append(jax.core.ShapedArray(shape, dtype))
            zero_outs.append(np.zeros(shape, dtype))
    n_params = len(in_names)
    all_in_names = list(in_names) + list(out_names)
    if partition_name is not None:
        all_in_names.append(partition_name)

    def _body(*args):
        operands = list(args)
        if partition_name is not None:
            operands.append(bass2jax.partition_id_tensor())
        outs = bass2jax._bass_exec_p.bind(
            *operands,
            out_avals=tuple(out_avals),
            in_names=tuple(all_in_names),
            out_names=tuple(out_names),
            lowering_input_output_aliases=(),
            sim_require_finite=True,
            sim_require_nnan=True,
            nc=nc,
        )
        return tuple(outs)

    devices = jax.devices()[:N_CORES]
    mesh = Mesh(np.asarray(devices), ("core",))
    nin = n_params + len(out_names)
    fn = jax.jit(shard_map(_body, mesh=mesh,
                           in_specs=(PartitionSpec("core"),) * nin,
                           out_specs=(PartitionSpec("core"),) * len(out_names),
                           check_rep=False))
    sh = NamedSharding(mesh, PartitionSpec("core"))
    dev_args = [
        jax.device_put(
            np.concatenate([np.asarray(in_maps[c][nm]) for c in range(N_CORES)],
                           axis=0), sh)
        for nm in in_names
    ] + [
        jax.device_put(np.zeros((N_CORES * z.shape[0], *z.shape[1:]), z.dtype), sh)
        for z in zero_outs
    ]

    def run_once():
        outs = fn(*dev_args)
        jax.block_until_ready(outs)
        return [
            {nm: np.asarray(outs[i]).reshape(N_CORES, *out_avals[i].shape)[c]
             for i, nm in enumerate(out_names)}
            for c in range(N_CORES)
        ]

    def time_iters(n):
        ts = []
        for _ in range(n):
            t0 = time.perf_counter()
            jax.block_until_ready(fn(*dev_args))
            ts.append(time.perf_counter() - t0)
        return ts

    return run_once, time_iters, None


def kernel(q, k, v, mask, Wq, bq, Wk, bk, Wv, bv, Wo, bo):
    global _last_key
    from concourse.bass_utils import run_bass_kernel_spmd

    q, k, v = (np.asarray(x) for x in (q, k, v))
    mask = np.asarray(mask)
    in_maps, nk = _host_prep(q, k, v, mask, np.asarray(Wq), np.asarray(bq),
                             np.asarray(Wk), np.asarray(bk), np.asarray(Wv),
                             np.asarray(bv), np.asarray(Wo))
    key = (nk, ("bq" in in_maps[0]), ("bk" in in_maps[0]), ("bv" in in_maps[0]))
    _last_key = key
    if key not in _cached:
        _cached[key] = _build_program(*key)
    nc = _cached[key]

    trace = bool(int(os.environ.get("KERNEL_TRACE", "0")))
    res = run_bass_kernel_spmd(nc, in_maps, list(range(N_CORES)), trace=trace)
    kernel.last_results = res

    y = np.zeros((T, D), np.float64)
    for i in range(N_CORES):
        y += res.results[i]["y"].astype(np.float64)
    y = (y + np.asarray(bo).astype(np.float64)).astype(np.float32)
    return y.reshape(B, L, D)


# revision 40
# speedup vs baseline: 1.8960x; 1.4745x over previous
"""Multi-head attention (B=2, L=2048, D=1024, H=16) on 8 Trainium2 NeuronCores.

Sharding: tensor-parallel over heads. Core c owns heads 2c, 2c+1, i.e. rows
[128c, 128c+128) of Wq/Wk/Wv and columns [128c, 128c+128) of Wo. Each core
computes Q projections for its 128 channels over all 4096 tokens, K/V
projections over the COMPACTED key set (see below), attention for its 2 heads
(both batches), and a partial out-projection y_c = attnO_c @ Wo[:, sl].T.
The host sums the 8 partials and adds bo (the gather/unshard step).

Key-padding-mask compaction: the mask is known on the host, and masked keys
contribute exactly 0 to softmax numerator and denominator (exp(-inf) == 0).
The host gathers only the unmasked key/value tokens per batch (~L/2 of them),
padding each batch to nk*128 tokens with zero columns that carry a -30000
exp-bias so they also contribute 0. This halves the dominant device work:
score matmuls, PV matmuls, and the softmax exp stream.

Device-side layout notes:
- The host pre-packs activations into the exact SBUF staging layout
  (128 partitions x [tile, chunk, token]) and weights into [128, c*128+e],
  so every load DMA is fully contiguous (full HBM bandwidth, one descriptor
  per partition) — no on-device transposes of inputs.
- Scores are computed transposed (S.T tiles [k,q]) so softmax P.T lands in
  SBUF ready to be the PV matmul's moving operand; softmax-over-partitions is
  avoided by skipping the max-subtraction (scores are ~N(0,1); exp cannot
  overflow fp32) and computing row sums r with a ones-column that rides along
  in the PV matmul (psum row 64 of each head's accumulator).
- The pad-token bias is 0 / -30000 per k-token, added per-partition by the
  ACT instruction, so pad keys exp to 0.0.
- One global software pipeline runs scores two k-tiles ahead of PV across
  attention-tile boundaries, so the exp stream never drains; projection and
  out-projection units are interleaved at fixed slots to fill the PE.
- The ACT (scalar) queue carries ONLY the exp stream; all DMAs are issued
  from the sync queue so a descriptor-generation stall never delays exp.
- Normalization 1/r is broadcast across partitions with a small matmul
  (compute engines cannot move data across partitions); the accumulators are
  copied out of PSUM immediately so the next tile's PV can reuse the banks.
- The two heads' normalized outputs are stacked into one [128, T] tile OT
  (head B's half moved with a SBUF->SBUF DMA, the only engine that can shift
  partitions), so the out-projection contracts all 128 channels in a single
  matmul per 512 output columns — half the PE rows of a split contraction.
- Output partials are written in bf16 (host sums in float64), halving the
  output DMA traffic.
"""

import os
import sys

for _p in ("/opt/trn_rl_repo", os.path.expanduser("~/.axon_site/_ro/trn_rl_repo")):
    if os.path.isdir(_p) and _p not in sys.path:
        sys.path.insert(0, _p)

import numpy as np

B = 2
L = 2048
D = 1024
T = B * L            # 4096 tokens
E = 128              # channels per core (2 heads x 64)
HD = 64              # head dim
N_CORES = 8
SCALE = 1.0 / 8.0    # 1/sqrt(HD)
MASK_BIAS = -30000.0

N_DC = D // 128      # 8 contraction chunks
N_QT = L // 512      # 4 query tiles per batch

_cached = {}
_last_key = None


def _kv_tiles(TKV):
    return [(o, min(512, TKV - o)) for o in range(0, TKV, 512)]


def _build_program(nk, has_bq, has_bk, has_bv, reps=1, mm_dt="f32r",
                   in_dt="bf16", out_dt="bf16", depth=2, pt_bufs=3,
                   prefetch=2, proj_cap=1, stg_bufs=6):
    import concourse.bacc as bacc
    import concourse.mybir as mybir
    import concourse.tile as tile

    KB = nk * 128            # padded kv tokens per batch
    TKV = B * KB             # total kv tokens
    G = B * nk               # 128-token kv groups
    kv_tiles = _kv_tiles(TKV)
    # interleave-slot boundaries: after scores(kt) for kt < bounds[ci]
    chunk_bounds = [c for c in ([3] + list(range(5, nk, 2))) if c < nk] + [nk]

    F32 = mybir.dt.float32
    AF = mybir.ActivationFunctionType
    ALU = mybir.AluOpType
    R = mybir.dt.float32r if mm_dt == "f32r" else mybir.dt.float32
    IDT = mybir.dt.bfloat16 if in_dt == "bf16" else F32
    ODT = mybir.dt.bfloat16 if out_dt == "bf16" else F32

    nc = bacc.Bacc("TRN2", target_bir_lowering=False, debug=False,
                   num_devices=N_CORES)

    def mm(out, lhsT, rhs, **kw):
        nc.tensor.matmul(out, lhsT, rhs, **kw)

    # activations pre-packed by the host into the staging layout
    qS = nc.dram_tensor("qS", [128, N_DC * T], IDT, kind="ExternalInput").ap()
    kS = nc.dram_tensor("kS", [128, N_DC * TKV], IDT, kind="ExternalInput").ap()
    vS = nc.dram_tensor("vS", [128, N_DC * TKV], IDT, kind="ExternalInput").ap()
    # weights pre-packed as w[p, c*128+e] = W.T[c*128+p, e]
    wq = nc.dram_tensor("wq", [128, D], IDT, kind="ExternalInput").ap()
    wk = nc.dram_tensor("wk", [128, D], IDT, kind="ExternalInput").ap()
    wv = nc.dram_tensor("wv", [128, D], IDT, kind="ExternalInput").ap()
    wo = nc.dram_tensor("wo", [E, D], R, kind="ExternalInput").ap()
    mbd = nc.dram_tensor("mb", [128, G], F32, kind="ExternalInput").ap()
    onesd = nc.dram_tensor("ones128", [128, 128], F32, kind="ExternalInput").ap()
    identd = nc.dram_tensor("ident", [128, 128], F32, kind="ExternalInput").ap()
    bias_d = {}
    if has_bq:
        bias_d["q"] = nc.dram_tensor("bq", [128, 1], F32, kind="ExternalInput").ap()
    if has_bk:
        bias_d["k"] = nc.dram_tensor("bk", [128, 1], F32, kind="ExternalInput").ap()
    if has_bv:
        bias_d["v"] = nc.dram_tensor("bv", [64, 2], F32, kind="ExternalInput").ap()
    yd = nc.dram_tensor("y", [T, D], ODT, kind="ExternalOutput").ap()

    with tile.TileContext(nc) as tc:
        import contextlib
        with contextlib.ExitStack() as ctx:
            const = ctx.enter_context(tc.tile_pool(name="const", bufs=1))
            big = ctx.enter_context(tc.tile_pool(name="big", bufs=1))
            stg = ctx.enter_context(tc.tile_pool(name="stg", bufs=stg_bufs))
            work = ctx.enter_context(tc.tile_pool(name="work", bufs=3))
            pt_pool = ctx.enter_context(tc.tile_pool(name="ptp", bufs=pt_bufs))
            # PSUM budget (8 banks of 2KB/partition):
            #   psst  (scores [128,1024])           2 bufs -> 4 banks
            #   psps  (proj ps/vtp/bc/y [128,512])  2 bufs -> 2 banks
            #   psacc (ot accumulators [65,512])    2 bufs -> 2 banks
            psst = ctx.enter_context(tc.tile_pool(name="psst", bufs=2, space="PSUM"))
            psps = ctx.enter_context(tc.tile_pool(name="psps", bufs=2, space="PSUM"))
            psacc = ctx.enter_context(tc.tile_pool(name="psacc", bufs=2, space="PSUM"))

            # ---- constants / weights (late-needed consts are loaded in the
            # rep-0 prologue so they stay off the startup critical path) ----
            w_sb = {}
            for nm, src in (("k", wk), ("q", wq), ("v", wv)):
                w_sb[nm] = const.tile([128, D], IDT, name=f"w{nm}_sb")
            nc.sync.dma_start(w_sb["k"][:], wk[:])
            nc.sync.dma_start(w_sb["q"][:], wq[:])
            mb_sb = const.tile([128, G], F32, name="mb_sb")
            b_sb = {}
            for nm in bias_d:
                b_sb[nm] = const.tile([128, 2] if nm == "v" else [128, 1], F32,
                                      name=f"b{nm}_sb")
            wo_sb = const.tile([128, D], R, name="wo_sb")
            ones_sb = const.tile([128, 128], F32, name="ones_sb")
            ident_sb = const.tile([128, 128], F32, name="ident_sb")

            # ---- persistent activations (feature-major) ----
            QT = big.tile([128, T], R, name="QT")     # [e, tok]
            KT = big.tile([128, TKV], R, name="KT")   # [e, tok]
            # V layout per 128-token group g: [V_headA(64) | 1 | V_headB(64) | 1]
            V = big.tile([128, G * 130], R, name="V")
            OT = big.tile([128, T], R, name="OT")     # [e, tok] normalized
            rr = big.tile([128, 2048], F32, name="rr")  # 1/r at partition 64

            carry = []   # leftover y units handed to the next rep's pump
            for rep in range(reps):
                srcs = {"q": qS, "k": kS, "v": vS}
                staged = {}

                def tile_geom(kind, i):
                    if kind == "q":
                        return i * 512, 512
                    return kv_tiles[i]

                def stage(kind, i, eng=None):
                    if (kind, i) in staged:
                        return
                    off, tw = tile_geom(kind, i)
                    s = stg.tile([128, N_DC * tw], IDT,
                                 name=f"{rep}_stg_{kind}_{i}", tag="stg",
                                 padded_shape=[128, N_DC * 512])
                    (eng or nc.sync).dma_start(
                        s[:], srcs[kind][:, N_DC * off:N_DC * (off + tw)])
                    staged[(kind, i)] = s

                def proj_unit(kind, i):
                    stage(kind, i)
                    s = staged.pop((kind, i))
                    off, tw = tile_geom(kind, i)
                    ps = psps.tile([128, tw], F32,
                                   name=f"{rep}_ps_{kind}_{i}", tag="ps",
                                   padded_shape=[128, 512])
                    for dc in range(N_DC):
                        mm(ps[:], w_sb[kind][:, dc * 128:(dc + 1) * 128],
                           s[:, dc * tw:(dc + 1) * tw],
                           start=(dc == 0), stop=(dc == N_DC - 1))
                    if kind in ("q", "k"):
                        dst = (QT if kind == "q" else KT)[:, off:off + tw]
                        if kind in b_sb:
                            nc.vector.tensor_scalar(dst, ps[:], b_sb[kind][:, 0:1],
                                                    None, ALU.add)
                        else:
                            nc.vector.tensor_copy(dst, ps[:])
                    else:
                        vs = work.tile([128, tw], F32, name=f"{rep}_vs_{i}",
                                       tag="vs", padded_shape=[128, 512])
                        nc.vector.tensor_copy(vs[:], ps[:])
                        for si in range(tw // 128):
                            vtp = psps.tile([128, 128], F32,
                                            name=f"{rep}_vtp_{i}_{si}", tag="ps",
                                            padded_shape=[128, 512])
                            nc.tensor.transpose(vtp[:],
                                                vs[:, si * 128:(si + 1) * 128],
                                                ident_sb[:])
                            g = off // 128 + si
                            nc.vector.tensor_copy(
                                V[:, g * 130:g * 130 + 64], vtp[:, 0:64])
                            nc.vector.tensor_copy(
                                V[:, g * 130 + 65:g * 130 + 129], vtp[:, 64:128])

                def y_unit(g, tail=False):
                    ys = work.tile([128, 1024], ODT, name=f"{rep}_ys_{g}", tag="ys")
                    for nn in range(2):
                        yp = psps.tile([128, 512], F32,
                                       name=f"{rep}_yp_{g}_{nn}", tag="ps")
                        mm(yp[:],
                           OT[:, g * 128:(g + 1) * 128],
                           wo_sb[:, nn * 512:(nn + 1) * 512],
                           start=True, stop=True)
                        dst = ys[:, nn * 512:(nn + 1) * 512]
                        # PSUM can only be drained by DVE/ACT; keep ACT clear
                        # for exp except in the tail where exp is done
                        if tail and nn:
                            nc.scalar.copy(dst, yp[:])
                        else:
                            nc.vector.tensor_copy(dst, yp[:])
                    nc.sync.dma_start(yd[g * 128:(g + 1) * 128, :], ys[:])

                # ---- global software-pipelined attention ----
                attn_order = [(b, qt) for b in range(B) for qt in range(N_QT)]
                ot_tiles = {}

                def scores(ai, b, qt, kt):
                    q0 = b * L + qt * 512
                    k0 = b * KB + kt * 128
                    g = b * nk + kt
                    s = psst.tile([128, 1024], F32,
                                  name=f"{rep}_st_{ai}_{kt}", tag="st")
                    for h in range(2):
                        mm(s[:, h * 512:(h + 1) * 512],
                           KT[h * 64:(h + 1) * 64, k0:k0 + 128],
                           QT[h * 64:(h + 1) * 64, q0:q0 + 512],
                           start=True, stop=True)
                    p = pt_pool.tile([128, 1024], R,
                                     name=f"{rep}_pt_{ai}_{kt}", tag="pt")
                    nc.scalar.activation(p[:], s[:], AF.Exp,
                                         bias=mb_sb[:, g:g + 1],
                                         scale=SCALE)
                    return p

                def pv(ai, b, qt, kt, p):
                    if kt == 0:
                        ot_tiles[ai] = [
                            psacc.tile([65, 512], F32,
                                       name=f"{rep}_ot{h}_{ai}", tag="acc")
                            for h in range(2)]
                    ot = ot_tiles[ai]
                    g = b * nk + kt
                    for h in range(2):
                        mm(ot[h][:],
                           V[:, g * 130 + 65 * h: g * 130 + 65 * (h + 1)],
                           p[:, h * 512:(h + 1) * 512],
                           start=(kt == 0), stop=(kt == nk - 1))
                    if kt == nk - 1:
                        norm_begin(ai, b, qt)

                norm_pend = []

                def norm_begin(ai, b, qt):
                    # free the PSUM accumulators ASAP: pull |r| and the head
                    # outputs into SBUF right after the last PV
                    ot = ot_tiles.pop(ai)
                    rb = (ai % 2) * 1024
                    otc = work.tile([64, 1024], F32,
                                    name=f"{rep}_otc_{ai}", tag="otc")
                    with nc.allow_low_precision(reason="feeds f32r matmul"):
                        for h in range(2):
                            nc.vector.reciprocal(
                                rr[64:65, rb + h * 512:rb + (h + 1) * 512],
                                ot[h][64:65, :])
                            nc.vector.tensor_copy(
                                otc[:, h * 512:(h + 1) * 512], ot[h][0:64, :])
                    norm_pend.append((ai, b, qt, otc))

                def norm_finish():
                    ai, b, qt, otc = norm_pend.pop(0)
                    q0 = b * L + qt * 512
                    rb = (ai % 2) * 1024
                    bcs = work.tile([64, 1024], F32,
                                    name=f"{rep}_bcs_{ai}", tag="bcs")
                    for h in range(2):
                        bc_ps = psps.tile([64, 512], F32,
                                          name=f"{rep}_bc{h}_{ai}", tag="ps",
                                          padded_shape=[128, 512])
                        mm(bc_ps[:], ones_sb[64:65, 0:64],
                           rr[64:65, rb + h * 512:rb + (h + 1) * 512],
                           start=True, stop=True)
                        nc.vector.tensor_copy(bcs[:, h * 512:(h + 1) * 512],
                                              bc_ps[:])
                    # head A lands directly on partitions 0:64
                    dstA = OT[0:64, q0:q0 + 512]
                    nc.vector.tensor_mul(dstA, otc[:, 0:512], bcs[:, 0:512])
                    if "v" in b_sb:
                        nc.vector.tensor_scalar(dstA, dstA, b_sb["v"][0:64, 0:1],
                                                None, ALU.add)
                    # head B needs a partition shift 0:64 -> 64:128 (DMA only)
                    otn = work.tile([64, 512], R,
                                    name=f"{rep}_otn_{ai}", tag="otn")
                    nc.vector.tensor_mul(otn[:], otc[:, 512:1024],
                                         bcs[:, 512:1024])
                    if "v" in b_sb:
                        nc.vector.tensor_scalar(otn[:], otn[:],
                                                b_sb["v"][0:64, 1:2],
                                                None, ALU.add)
                    nc.sync.dma_start(OT[64:128, q0:q0 + 512], otn[:])
                    completed_attn[0] = ai + 1

                # ---- dependency-aware interleaved emission ----
                emitted = set()

                def emit(u):
                    if u in emitted:
                        return
                    emitted.add(u)
                    if u[0] == "y":
                        y_unit(u[1])
                    else:
                        proj_unit(u[0], u[1])

                def ensure_kv(colbound):
                    for i, (off, tw) in enumerate(kv_tiles):
                        if off < colbound:
                            emit(("k", i))
                            emit(("v", i))

                prio = []
                nkv = len(kv_tiles)
                qv = [("q", i) for i in range(1, 8)]
                kvv = [x for i in range(1, nkv) for x in (("k", i), ("v", i))]
                while qv or kvv:
                    if kvv:
                        prio.append(kvv.pop(0))
                        prio.append(kvv.pop(0))
                    if qv:
                        prio.append(qv.pop(0))
                for g in range(32):
                    prio.append(("y", g))

                completed_attn = [0]

                def pump(proj_cap=proj_cap, y_cap=2):
                    np_, ny = 0, 0
                    # previous rep's leftover y units fill the early slots
                    # (their OT columns are valid until this rep's normalize)
                    while carry and ny < y_cap:
                        g, fn = carry.pop(0)
                        fn()
                        ny += 1
                    picks = []
                    for u in prio:
                        if u in emitted:
                            continue
                        if u[0] == "y":
                            # one-tile delay so the y matmul's OT stationary
                            # never waits on the just-emitted normalize chain
                            if u[1] // 4 >= completed_attn[0] - 1:
                                break
                            if ny < y_cap and np_ + ny < 2:
                                picks.append(u)
                                ny += 1
                        elif np_ < proj_cap:
                            picks.append(u)
                            np_ += 1
                        if np_ + ny >= 2:
                            break
                    for u in picks:
                        prio.remove(u)
                        emit(u)
                    # prefetch staging for upcoming projection units
                    ahead = 0
                    for u in prio:
                        if u[0] in ("k", "v", "q") and u not in emitted:
                            stage(u[0], u[1])
                            ahead += 1
                            if ahead >= prefetch:
                                break

                def slot():
                    # pumped PE work lands between the last pv and the bc
                    # matmuls, covering the reciprocal's DVE latency
                    pump()
                    while norm_pend:
                        # safety: a carried y reading OT columns this tile is
                        # about to overwrite must be emitted first
                        ai_next = norm_pend[0][0]
                        for c in [c for c in carry if c[0] // 4 == ai_next]:
                            carry.remove(c)
                            c[1]()
                        norm_finish()

                # startup: k0/q0/v0 issue in parallel on three queues; the
                # late-needed consts follow on the scalar queue (idle before
                # the first exp)
                stage("k", 0)
                stage("q", 0, eng=nc.scalar)
                stage("v", 0, eng=nc.gpsimd)
                if rep == 0:
                    nc.scalar.dma_start(w_sb["v"][:], wv[:])
                    nc.scalar.dma_start(mb_sb[:], mbd[:])
                    for nm, src in bias_d.items():
                        nc.scalar.dma_start(b_sb[nm][:], src[:])
                    nc.scalar.dma_start(ident_sb[:], identd[:])
                    nc.scalar.dma_start(ones_sb[:], onesd[:])
                    nc.scalar.dma_start(wo_sb[:], wo[:])
                    for g in range(G):
                        nc.gpsimd.memset(
                            V.bitcast(F32)[:, g * 130 + 64:g * 130 + 65], 1.0)
                        nc.gpsimd.memset(
                            V.bitcast(F32)[:, g * 130 + 129:g * 130 + 130], 1.0)

                DEPTH = depth
                pend = []

                def flush_pv():
                    a = pend.pop(0)
                    pv(*a)

                for ai, (b, qt) in enumerate(attn_order):
                    emit(("q", b * N_QT + qt))
                    lo = 0
                    for bound in chunk_bounds:
                        ensure_kv(b * KB + bound * 128)
                        for kt in range(lo, bound):
                            pend.append((ai, b, qt, kt, scores(ai, b, qt, kt)))
                            if len(pend) > DEPTH:
                                flush_pv()
                        lo = bound
                        slot()
                while pend:
                    flush_pv()
                slot()
                # leftover y units: hand to the next rep's pump (their OT
                # columns stay valid until that rep's normalize overwrites
                # them), or drain in the tail on the final rep
                leftover = [u for u in prio if u not in emitted]
                if rep < reps - 1:
                    for u in leftover:
                        if u[0] == "y":
                            emitted.add(u)
                            g = u[1]
                            carry.append((g, lambda g=g, f=y_unit: f(g)))
                        else:
                            emit(u)
                else:
                    for u in leftover:
                        if u[0] == "y":
                            emitted.add(u)
                            y_unit(u[1], tail=True)
                        else:
                            emit(u)

    nc.compile()
    return nc


def _host_prep(q, k, v, mask, Wq, bq, Wk, bk, Wv, bv, Wo, in_dt="bf16"):
    """Build the per-core input maps. Returns (in_maps, nk)."""
    import ml_dtypes
    f32 = np.float32
    idt = ml_dtypes.bfloat16 if in_dt == "bf16" else f32

    unm = ~np.asarray(mask)
    U = unm.sum(axis=1).astype(np.int64)          # unmasked keys per batch
    nk = int(max(1, -(-int(U.max()) // 128)))
    KB = nk * 128
    TKV = B * KB
    G = B * nk

    def pack_act(x_tok):
        """[ntok, D] -> [128, N_DC*ntok] staging layout: tiles of <=512
        tokens, inner [chunk, token] per partition."""
        ntok = x_tok.shape[0]
        blocks = []
        for off in range(0, ntok, 512):
            tw = min(512, ntok - off)
            blk = x_tok[off:off + tw].reshape(tw, N_DC, 128)
            blocks.append(np.transpose(blk, (2, 1, 0)).reshape(128, N_DC * tw))
        return np.ascontiguousarray(np.concatenate(blocks, axis=1).astype(idt))

    qS = pack_act(np.asarray(q).reshape(T, D))
    kc = np.zeros((TKV, D), f32)
    vc = np.zeros((TKV, D), f32)
    for b in range(B):
        idx = np.nonzero(unm[b])[0]
        kc[b * KB:b * KB + len(idx)] = k[b, idx]
        vc[b * KB:b * KB + len(idx)] = v[b, idx]
    kS = pack_act(kc)
    vS = pack_act(vc)

    mb = np.full((128, G), f32(MASK_BIAS), f32)
    for b in range(B):
        for t in range(nk):
            n = min(128, max(0, int(U[b]) - t * 128))
            mb[:n, b * nk + t] = 0.0
    ones128 = np.ones((128, 128), f32)
    ident = np.eye(128, dtype=f32)

    def pack_w(W_slice):
        """[E, D] torch-layout slice -> [128, c*128+e] = W.T[c*128+p, e]."""
        wT = W_slice.T.reshape(N_DC, 128, E)          # [c, p, e]
        return np.ascontiguousarray(
            np.transpose(wT, (1, 0, 2)).reshape(128, N_DC * E).astype(idt))

    in_maps = []
    for c in range(N_CORES):
        sl = slice(c * E, (c + 1) * E)
        m = {
            "qS": qS, "kS": kS, "vS": vS,
            "wq": pack_w(Wq[sl, :]),
            "wk": pack_w(Wk[sl, :]),
            "wv": pack_w(Wv[sl, :]),
            "wo": np.ascontiguousarray(Wo[:, sl].T.astype(f32)),
            "mb": mb, "ones128": ones128, "ident": ident,
        }
        if np.any(bq):
            m["bq"] = np.ascontiguousarray(bq[sl].astype(f32).reshape(128, 1))
        if np.any(bk):
            m["bk"] = np.ascontiguousarray(bk[sl].astype(f32).reshape(128, 1))
        if np.any(bv):
            m["bv"] = np.ascontiguousarray(bv[sl].astype(f32).reshape(2, 64).T)
        in_maps.append(m)
    return in_maps, nk


def _make_timed_runner(nc, in_maps):
    """Build a reusable jitted runner for `nc` (no output donation — the
    program writes every output element, so uninit result buffers are fine).
    Returns (run_once() -> per-core outputs as numpy, time_iters(n) -> [sec])."""
    import jax
    import time
    import concourse.mybir as mybir
    from concourse import bass2jax
    from jax.experimental.shard_map import shard_map
    from jax.sharding import Mesh, NamedSharding, PartitionSpec

    bass2jax.install_neuronx_cc_hook()

    partition_name = nc.partition_id_tensor.name if nc.partition_id_tensor else None
    in_names, out_names, out_avals, zero_outs = [], [], [], []
    for alloc in nc.m.functions[0].allocations:
        if not isinstance(alloc, mybir.MemoryLocationSet):
            continue
        name = alloc.memorylocations[0].name
        if alloc.kind == "ExternalInput":
            if name != partition_name:
                in_names.append(name)
        elif alloc.kind == "ExternalOutput":
            shape = tuple(alloc.tensor_shape)
            dtype = mybir.dt.np(alloc.dtype)
            out_names.append(name)
            out_avals.append(jax.core.ShapedArray(shape, dtype))
            zero_outs.append(np.zeros(shape, dtype))
    n_params = len(in_names)
    all_in_names = list(in_names) + list(out_names)
    if partition_name is not None:
        all_in_names.append(partition_name)

    def _body(*args):
        operands = list(args)
        if partition_name is not None:
            operands.append(bass2jax.partition_id_tensor())
        outs = bass2jax._bass_exec_p.bind(
            *operands,
            out_avals=tuple(out_avals),
            in_names=tuple(all_in_names),
            out_names=tuple(out_names),
            lowering_input_output_aliases=(),
            sim_require_finite=True,
            sim_require_nnan=True,
            nc=nc,
        )
        return tuple(outs)

    devices = jax.devices()[:N_CORES]
    mesh = Mesh(np.asarray(devices), ("core",))
    nin = n_params + len(out_names)
    fn = jax.jit(shard_map(_body, mesh=mesh,
                           in_specs=(PartitionSpec("core"),) * nin,
                           out_specs=(PartitionSpec("core"),) * len(out_names),
                           check_rep=False))
    sh = NamedSharding(mesh, PartitionSpec("core"))
    dev_args = [
        jax.device_put(
            np.concatenate([np.asarray(in_maps[c][nm]) for c in range(N_CORES)],
                           axis=0), sh)
        for nm in in_names
    ] + [
        jax.device_put(np.zeros((N_CORES * z.shape[0], *z.shape[1:]), z.dtype), sh)
        for z in zero_outs
    ]

    def run_once():
        outs = fn(*dev_args)
        jax.block_until_ready(outs)
        return [
            {nm: np.asarray(outs[i]).reshape(N_CORES, *out_avals[i].shape)[c]
             for i, nm in enumerate(out_names)}
            for c in range(N_CORES)
        ]

    def time_iters(n):
        ts = []
        for _ in range(n):
            t0 = time.perf_counter()
            jax.block_until_ready(fn(*dev_args))
            ts.append(time.perf_counter() - t0)
        return ts

    return run_once, time_iters, None


def kernel(q, k, v, mask, Wq, bq, Wk, bk, Wv, bv, Wo, bo):
    global _last_key
    from concourse.bass_utils import run_bass_kernel_spmd

    q, k, v = (np.asarray(x) for x in (q, k, v))
    mask = np.asarray(mask)
    in_maps, nk = _host_prep(q, k, v, mask, np.asarray(Wq), np.asarray(bq),
                             np.asarray(Wk), np.asarray(bk), np.asarray(Wv),
                             np.asarray(bv), np.asarray(Wo))
    key = (nk, ("bq" in in_maps[0]), ("bk" in in_maps[0]), ("bv" in in_maps[0]))
    _last_key = key
    if key not in _cached:
        _cached[key] = _build_program(*key)
    nc = _cached[key]

    trace = bool(int(os.environ.get("KERNEL_TRACE", "0")))
    res = run_bass_kernel_spmd(nc, in_maps, list(range(N_CORES)), trace=trace)
    kernel.last_results = res

    y = np.zeros((T, D), np.float64)
    for i in range(N_CORES):
        y += res.results[i]["y"].astype(np.float64)
    y = (y + np.asarray(bo).astype(np.float64)).astype(np.float32)
    return y.reshape(B, L, D)
